# revision 1
# baseline (speedup 1.0000x reference)
"""Trainium2 Bass kernel for nn_BestChangeLayer (GoL pattern search).

Math: for each batch b, the 7x7 window W of x at (ry,rx) gets its center 3x3
replaced by each of 512 patterns p; one Game-of-Life step runs and the inner
5x5 is compared with the target window tw. Since GoL is new = [s==3]+[s==2]*c
(s = 8-neighbor sum) and the error is linear in new, the 512-pattern sweep
collapses to errors = G @ T with a per-batch feature matrix G (128 features)
and a precomputed constant table T (128 x 512):
  - split s = S_fix(b,cell) + S_pat(p,cell); one-hot S_fix over v=0..3
    (s>=4 -> new=0), pair [S_fix==v]*w (and *center for ring cells) with
    pattern-side indicators [S_pat==3-v] / [S_pat==2-v].
  - w = 1-2*tw folds |new-tw| = w*new + tw; the +tw sum rides along as 25
    extra all-ones table rows so E equals the reference errors EXACTLY
    (all integer arithmetic, exact in fp32/bf16).
Then seeded = E + 0.5*noise bit-matches the reference, argmin (first index)
via min -> is_equal mask -> masked iota -> min, bits of the argmin via int
shifts, patched into the x passthrough tile.

Sharding: pure data parallel, batch 1024 = 8 cores x 128 rows.
"""

import os
import sys

import numpy as np

for _p in ("/opt/trn_rl_repo", "/root/.axon_site/_ro/trn_rl_repo"):
    if os.path.isdir(_p) and _p not in sys.path:
        sys.path.insert(0, _p)

import ml_dtypes  # noqa: E402

import concourse.bass as bass  # noqa: E402,F401
import concourse.mybir as mybir  # noqa: E402
import concourse.tile as tile  # noqa: E402
from concourse import bacc  # noqa: E402
from concourse.bass_utils import run_bass_kernel_spmd  # noqa: E402
from concourse.masks import make_identity  # noqa: E402

N_CORES = 8
B_TOTAL = 1024
B = B_TOTAL // N_CORES  # 128 batch rows per core
H = W = 25
NPAT = 512

# ---------------------------------------------------------------------------
# Host-side constant tables (pure functions of the 512 patterns / geometry).
# ---------------------------------------------------------------------------
# cells: 5x5 output cells at window pos (i+1, j+1), ring (center from W)
# first. Engine SBUF APs must start at partition 0/32/64/96, so each v-block
# occupies a 32-partition slot; invalid (cell,v) table rows are naturally 0.
G1_VS = [3, 2, 1, 0]   # Ga slots 0,32,64,96 : w*[S_fix==v], 25 cells each
G2_VS = [2, 1, 0]      # Gb slots 0,32,64   : w*c*[S_fix==v], 16 ring cells


def _cell_order():
    corner, edgeadj, midedge, inner = [], [], [], []
    for i in range(5):
        for j in range(5):
            r, c = i + 1, j + 1
            nr = len({r - 1, r, r + 1} & {2, 3, 4})
            ncc = len({c - 1, c, c + 1} & {2, 3, 4})
            if 2 <= r <= 4 and 2 <= c <= 4:
                inner.append((i, j))
            elif nr * ncc == 1:
                corner.append((i, j))
            elif nr * ncc == 2:
                edgeadj.append((i, j))
            else:
                midedge.append((i, j))
    return corner + edgeadj + midedge + inner


CELLS = _cell_order()


def _build_tables():
    ints = np.arange(NPAT)
    shifts = np.arange(8, -1, -1)
    pats = ((ints[:, None] >> shifts[None, :]) & 1).astype(np.float32).reshape(NPAT, 3, 3)

    n8_fix, n8_pat, centers, is_inner = [], [], [], []
    for (i, j) in CELLS:
        r, c = i + 1, j + 1
        nb_fix, nb_pat = [], []
        for dr in (-1, 0, 1):
            for dc in (-1, 0, 1):
                if dr == 0 and dc == 0:
                    continue
                u, v = r + dr, c + dc
                (nb_pat if (2 <= u <= 4 and 2 <= v <= 4) else nb_fix).append((u, v))
        n8_fix.append(nb_fix)
        n8_pat.append(nb_pat)
        centers.append((r, c))
        is_inner.append(2 <= r <= 4 and 2 <= c <= 4)

    # M_const (74, 80): acts on the transposed staging tile [W(49) | w(25)]
    # where w = 1-2*tw is precomputed batch-major on the front.
    #   cols  0..24: S_fix   (sum of non-pattern neighbors from W, CELL order)
    #   cols 32..56: w permuted to CELL order (passthrough)
    #   cols 64..79: c_ring  (W center value for the 16 ring cells)
    # (quarter-aligned so downstream engine reads start at 0/32/64)
    M_const = np.zeros((74, 80), np.float32)
    for ci, nb in enumerate(n8_fix):
        for (u, v) in nb:
            M_const[u * 7 + v, ci] = 1.0
    for ci, (i, j) in enumerate(CELLS):
        M_const[49 + i * 5 + j, 32 + ci] = 1.0
    for ci in range(16):
        r, c = centers[ci]
        M_const[r * 7 + c, 64 + ci] = 1.0

    S_pat = np.zeros((NPAT, 25), np.float32)
    C_pat = np.zeros((NPAT, 25), np.float32)
    for ci in range(25):
        for (u, v) in n8_pat[ci]:
            S_pat[:, ci] += pats[:, u - 2, v - 2]
        if is_inner[ci]:
            r, c = centers[ci]
            C_pat[:, ci] = pats[:, r - 2, c - 2]

    # Ta (128,512): G1 v-blocks in 32-row slots; pad rows zero.
    Ta = np.zeros((128, NPAT), np.float32)
    for k, v in enumerate(G1_VS):
        for ci in range(25):
            t1 = (S_pat[:, ci] == 3 - v).astype(np.float32)
            if is_inner[ci]:
                t1 = t1 + C_pat[:, ci] * (S_pat[:, ci] == 2 - v)
            Ta[32 * k + ci] = t1
    # Tb (128,512): G2 v-blocks (ring cells) in 32-row slots + tw rows
    # (all-ones, the +sum(tw) term) in the last quarter (96..120).
    Tb = np.zeros((128, NPAT), np.float32)
    for k, v in enumerate(G2_VS):
        for ci in range(16):
            Tb[32 * k + ci] = (S_pat[:, ci] == 2 - v).astype(np.float32)
    Tb[96:121] = 1.0
    # Two constant buffers: tiny geometry matrix (loaded first, gates the
    # S matmul) and the big pattern tables (needed later, by the E matmuls).
    CONST_T = np.zeros((128, 2 * NPAT), np.float32)
    CONST_T[:, :NPAT] = Ta
    CONST_T[:, NPAT:] = Tb
    return M_const, CONST_T


def _mod_runs(start, length, m):
    """Split indices [(start+k)%m for k in range(length)] into consecutive runs.

    Returns [(dst_off, src_start, run_len)].
    """
    idx = [(start + k) % m for k in range(length)]
    runs, k = [], 0
    while k < length:
        s0 = idx[k]
        n = 1
        while k + n < length and idx[k + n] == s0 + n:
            n += 1
        runs.append((k, s0, n))
        k += n
    return runs


# ---------------------------------------------------------------------------
# Kernel builder
# ---------------------------------------------------------------------------
_CACHE = {}

F32 = mybir.dt.float32
BF16 = mybir.dt.bfloat16
I32 = mybir.dt.int32


def _build(ry, rx):
    assert 0 <= ry <= H - 3 and 0 <= rx <= W - 3, (ry, rx)
    M_const, CONST_T = _build_tables()
    bf = ml_dtypes.bfloat16

    nc = bacc.Bacc(None, target_bir_lowering=False)
    x_h = nc.dram_tensor("x", [B, H * W], F32, kind="ExternalInput")
    t_h = nc.dram_tensor("target", [B, H * W], F32, kind="ExternalInput")
    n_h = nc.dram_tensor("noise", [B, NPAT], F32, kind="ExternalInput")
    o_h = nc.dram_tensor("out", [B, H * W], F32, kind="ExternalOutput")
    mconst_h = nc.inline_tensor(M_const.astype(bf), "mconst")
    constt_h = nc.inline_tensor(CONST_T.astype(bf), "consttab")

    OP = mybir.AluOpType

    with tile.TileContext(nc) as tc:
        with (
            tc.tile_pool(name="sb", bufs=1) as sb,
            tc.tile_pool(name="ps", bufs=1, space="PSUM") as ps,
        ):
            ident = sb.tile([128, 128], BF16)
            make_identity(nc, ident[:])

            # --- DMA front. Window loads are contiguous ROW BANDS (cheap
            # single-descriptor dispatch); the 2D window extraction happens in
            # the SBUF staging copies. sync: mconst, w_rows, tables, [outC];
            # scalar: t_rows, noise, x, early-out stores.
            # x rows (ry-2..ry+4)%H as <=2 contiguous bands -> w_rows (B,175)
            w_rows = sb.tile([B, 7 * W], F32)
            for (di, si, nr) in _mod_runs(ry - 2, 7, H):
                nc.sync.dma_start(
                    out=w_rows[:, di * W:(di + nr) * W],
                    in_=x_h[:, si * W:(si + nr) * W],
                )
            mconst = sb.tile([74, 80], BF16)
            nc.sync.dma_start(out=mconst[:], in_=mconst_h[:, :])
            cbuf = sb.tile([128, 2 * NPAT], BF16)
            nc.sync.dma_start(out=cbuf[:], in_=constt_h[:, :])

            # target rows (ry-1..ry+3)%H -> t_rows (B,125)
            t_rows = sb.tile([B, 5 * W], F32)
            for (di, si, nr) in _mod_runs(ry - 1, 5, H):
                nc.scalar.dma_start(
                    out=t_rows[:, di * W:(di + nr) * W],
                    in_=t_h[:, si * W:(si + nr) * W],
                )
            noise = sb.tile([B, NPAT], F32)
            nc.scalar.dma_start(out=noise[:], in_=n_h[:, :])
            x_tile = sb.tile([B, H * W], F32)
            nc.scalar.dma_start(out=x_tile[:], in_=x_h[:, :])

            # --- staging [W 7x7 | w 5x5] bf16 (w = 1-2*tw) ---
            # high_priority: keep these at the head of their engine streams so
            # later ops with not-yet-ready deps can't head-of-line block them.
            stage = sb.tile([B, 74], BF16)
            with tc.high_priority():
                ws3 = stage[:, 0:49].rearrange("b (h w) -> b h w", h=7)
                wr3 = w_rows[:].rearrange("b (h w) -> b h w", h=7)
                for (dj, sj, ncc) in _mod_runs(rx - 2, 7, W):
                    nc.vector.tensor_copy(
                        out=ws3[:, :, dj:dj + ncc], in_=wr3[:, :, sj:sj + ncc])
                ts3 = stage[:, 49:74].rearrange("b (h w) -> b h w", h=5)
                tr3 = t_rows[:].rearrange("b (h w) -> b h w", h=5)
                for (dj, sj, ncc) in _mod_runs(rx - 1, 5, W):
                    # same engine as the W-copy: cross-engine writers of one
                    # tile serialize with sem latency
                    nc.vector.tensor_scalar(
                        ts3[:, :, dj:dj + ncc], tr3[:, :, sj:sj + ncc],
                        -2.0, 1.0, OP.mult, OP.add,
                    )

            # --- transpose staging -> (74, B) bf16, then geometry matmul ---
            stageT_ps = ps.tile([74, B], BF16)
            nc.tensor.transpose(out=stageT_ps[:], in_=stage[:], identity=ident[:])
            stageT = sb.tile([74, B], BF16)
            with tc.high_priority(offset=1000):
                nc.vector.tensor_copy(out=stageT[:], in_=stageT_ps[:])

            # all (80, B): rows 0..24 S_fix, 32..56 w (CELL order), 64..79 c_ring
            s_ps = ps.tile([80, B], F32)
            nc.tensor.matmul(s_ps[:], mconst[:], stageT[:], start=True, stop=True)
            # keep PE busy so its clock ramps before the E matmuls (HAM warmup)
            warm_ps = ps.tile([128, B], BF16)
            for _ in range(4):
                nc.tensor.transpose(out=warm_ps[:], in_=ident[:], identity=ident[:])
            # PSUM reads stay on one engine (DVE); two-SBUF-operand ops must
            # share a base partition (walrus NCC_IBIR297), so pair PSUM+SBUF
            # or base-0+base-0 operands only.
            # cols 0..B: copy of s_ps; cols B..2B rows 0..24: w duplicated at
            # partition base 0 so Ga's two SBUF operands share a base.
            s_sb = sb.tile([80, 2 * B], F32)
            nc.vector.tensor_copy(out=s_sb[:, 0:B], in_=s_ps[:])
            nc.vector.tensor_copy(out=s_sb[0:25, B:2 * B], in_=s_sb[32:57, 0:B])
            # wc = c_ring * w_ring  [DVE; PSUM in0 + SBUF in1]
            wc = sb.tile([16, B], F32)
            nc.vector.scalar_tensor_tensor(
                out=wc[:], in0=s_ps[64:80, :], scalar=1.0, in1=s_sb[32:48, 0:B],
                op0=OP.mult, op1=OP.mult,
            )
            # Ga (128,B): w*[S_fix==v] in 32-row slots  [DVE; PSUM in0]
            Ga = sb.tile([128, B], BF16)
            nc.vector.memset(Ga[:], 0.0)
            for k, v in enumerate(G1_VS):
                nc.vector.scalar_tensor_tensor(
                    out=Ga[32 * k:32 * k + 25, :], in0=s_sb[0:25, 0:B], scalar=float(v),
                    in1=s_sb[0:25, B:2 * B], op0=OP.is_equal, op1=OP.mult,
                )
            # Gb (128,B): w*c*[S_fix==v] slots 0..2 + tw rows at 96..120
            Gb = sb.tile([128, B], BF16)
            nc.gpsimd.memset(Gb[:], 0.0)
            for k, v in enumerate(G2_VS):
                nc.vector.scalar_tensor_tensor(
                    out=Gb[32 * k:32 * k + 16, :], in0=s_sb[0:16, 0:B], scalar=float(v),
                    in1=wc[:], op0=OP.is_equal, op1=OP.mult,
                )
            # tw = Copy(-0.5*w + 0.5) rows (the +sum(tw) term)  [ACT]
            nc.scalar.activation(
                Gb[96:121, :], s_sb[32:57, 0:B], mybir.ActivationFunctionType.Copy,
                bias=0.5, scale=-0.5,
            )

            # early stores of the non-patched rows (scalar queue, after x;
            # overlaps the tail chain)
            if ry > 0:
                nc.scalar.dma_start(out=o_h[:, 0:ry * W], in_=x_tile[:, 0:ry * W])
            if ry + 3 < H:
                nc.scalar.dma_start(
                    out=o_h[:, (ry + 3) * W:], in_=x_tile[:, (ry + 3) * W:])

            # E (B, 512) = Ga^T@Ta + Gb^T@Tb' == reference errors exactly.
            # Split per 32-row slot so PE consumes each G slot as soon as the
            # DVE finishes it instead of waiting for the whole tile.
            E_ps = ps.tile([B, NPAT], F32)
            nc.tensor.matmul(E_ps[:], Ga[:], cbuf[:, 0:NPAT], start=True, stop=False)
            nc.tensor.matmul(E_ps[:], Gb[:], cbuf[:, NPAT:2 * NPAT], start=False, stop=True)

            # negseed = -(E + 0.5*noise) = (noise * -0.5) - E ; argmax == argmin
            negseed = sb.tile([B, NPAT], F32)
            nc.vector.scalar_tensor_tensor(
                out=negseed[:], in0=noise[:], scalar=-0.5, in1=E_ps[:],
                op0=OP.mult, op1=OP.subtract,
            )
            mx8 = sb.tile([B, 8], F32)
            nc.vector.max(out=mx8[:], in_=negseed[:])
            idx8 = sb.tile([B, 8], mybir.dt.uint32)
            nc.vector.max_index(out=idx8[:], in_max=mx8[:], in_values=negseed[:])

            # bits (B,9): bit_j = (idx >> (8-j)) & 1  (idx free-broadcast to 9)
            U32 = mybir.dt.uint32
            sh = sb.tile([B, 9], U32)
            nc.gpsimd.iota(sh[:], pattern=[[-1, 9]], base=8, channel_multiplier=0)
            shd = sb.tile([B, 9], U32)
            nc.vector.tensor_tensor(
                out=shd[:], in0=idx8[:, 0:1].to_broadcast([B, 9]), in1=sh[:],
                op=OP.logical_shift_right,
            )
            bit = sb.tile([B, 9], U32)
            nc.vector.tensor_scalar(bit[:], shd[:], 1, None, OP.bitwise_and)
            x3v = x_tile[:].rearrange("b (h w) -> b h w", h=H)
            nc.vector.tensor_copy(
                out=x3v[:, ry:ry + 3, rx:rx + 3],
                in_=bit[:].rearrange("b (h w) -> b h w", h=3),
            )
            nc.sync.dma_start(
                out=o_h[:, ry * W:(ry + 3) * W], in_=x_tile[:, ry * W:(ry + 3) * W])

    nc.finalize()
    return nc


def _get(ry, rx):
    key = (ry, rx)
    if key not in _CACHE:
        _CACHE[key] = _build(ry, rx)
    return _CACHE[key]


def kernel_with_results(x, target, noise, ry, rx, trace=False):
    x = np.ascontiguousarray(np.asarray(x, dtype=np.float32))
    target = np.ascontiguousarray(np.asarray(target, dtype=np.float32))
    noise = np.ascontiguousarray(np.asarray(noise, dtype=np.float32))
    ry, rx = int(ry), int(rx)
    Btot = x.shape[0]
    assert Btot == B_TOTAL and x.shape == (Btot, 1, H, W), x.shape

    nc = _get(ry, rx)
    xs = x.reshape(Btot, H * W)
    ts = target.reshape(Btot, H * W)
    in_maps = [
        {
            "x": xs[c * B:(c + 1) * B],
            "target": ts[c * B:(c + 1) * B],
            "noise": noise[c * B:(c + 1) * B],
        }
        for c in range(N_CORES)
    ]
    res = run_bass_kernel_spmd(nc, in_maps, core_ids=list(range(N_CORES)), trace=trace)
    out = np.concatenate([res.results[c]["out"] for c in range(N_CORES)], axis=0)
    return out.reshape(Btot, 1, H, W).astype(np.float32), res


def kernel(x, target, noise, ry, rx):
    out, _ = kernel_with_results(x, target, noise, ry, rx)
    return out

